# revision 11
# baseline (speedup 1.0000x reference)
"""Trainium2 Bass kernel for nn_LinearEmbed (GINE message passing + all-pairs edge embed).

Sharding: data-parallel over graphs. 64 graphs -> 8 cores x 8 graphs.
Cross-core coupling: batchnorm statistics (AllReduce of [128,2] per layer).

Layout conventions (per core, G_loc=8 graphs, 512 nodes, 4096 edges):
  feature-major: [H=128 partitions, rows free]  (hT, u1T, ...)
  edge-major:    [128 edge partitions, H free]  (messages m, negu2, ec)
All matmuls in bf16 (f32 PSUM accumulate).

v2 notes:
  - bond encoder folded into u1 / ec weights on host (wu1 = bond_W @ gbm_W1,
    w3e = bond_W @ mlp_W1[2H:3H] + bias row): eT / eTb never materialized.
  - all u1 layers + negated-u2 (edge MLP part of messages) + ec emitted
    inside layer-0's collective window; layers 1-3 then only run the
    h-dependent gather matmuls and a 2-op relu  m = max(g, -u2) - (-u2).
  - const DMAs split across the SP / Activation / Pool DGE queues.
  - eib selector DMAs batched per 8-band group on the Pool queue.
  - per-band A' transposes replaced by one node-major A' (4 matmuls) +
    cross-partition vector copies.
"""

import os
import numpy as np
import ml_dtypes

import concourse.bass as bass
import concourse.mybir as mybir
import concourse.tile as tile
from concourse.vector_clock import ScopedClock
from concourse.bass_utils import run_bass_kernel_spmd

# problem constants
G, NP, EP, H = 64, 64, 512, 128
IN_F, EDGE_F, L = 32, 16, 4
BN_EPS = 1e-5
N_CORES = 8
G_LOC = G // N_CORES          # 8 graphs per core
N_LOC = G_LOC * NP            # 512 nodes
E_LOC = G_LOC * EP            # 4096 edges
NB = G_LOC * 8                # 64 bands per core (8 i-bands per graph)
BS = 96                       # band slot budget (max edges per band)
NTOT = float(G * NP)          # batchnorm population

f32 = mybir.dt.float32
bf16 = mybir.dt.bfloat16
AX = mybir.AxisListType
ALU = mybir.AluOpType
ACTF = mybir.ActivationFunctionType

bf = ml_dtypes.bfloat16


def _to_bf16(a):
    return np.asarray(a, dtype=np.float32).astype(bf)


class _SplitDrainTC(tile.TileContext):
    """Tail drain in this walrus build accepts only one sync wait; split the
    global-clock waits across multiple drain instructions."""

    def _drain_and_barrier(self, tick_clock, wait_clock):
        drain_inst = self.nc.sync.drain()
        wait_clock.add_sem_waits(
            drain_inst.ins, ScopedClock({None: tick_clock.global_clock})
        )
        si = drain_inst.ins.sync_info
        waits = list(si.on_wait or [])
        if len(waits) > 1:
            si.on_wait = [waits[0]]
            for w in waits[1:]:
                extra = self.nc.sync.drain()
                extra.ins.sync_info = mybir.SyncInfo(on_wait=[w], on_update=[])
        self.nc.all_engine_barrier()
        assert self.sems is not None
        popped = self.nc._tile_sem_poison_stack.pop()
        assert popped is self._sem_poison
        self.nc.clear_and_free_semaphores(list(self.sems.allocated().values()))
        self.nc.all_engine_barrier()


# ---------------------------------------------------------------------------
# host-side preprocessing: shard + sort + one ndarray per SBUF constant
# ---------------------------------------------------------------------------

def _prep_core(c, x, edge_attr, src, dst, weights):
    g0 = c * G_LOC
    lo, hi = g0 * NP, (g0 + G_LOC) * NP
    mask = (src >= lo) & (src < hi)
    esel = np.nonzero(mask)[0]
    assert ((dst[esel] >= lo) & (dst[esel] < hi)).all(), "cross-shard edge"

    # stable sort local edges by (graph, band)
    s_loc = src[esel] - lo
    band_key = (s_loc // NP) * 8 + (s_loc % NP) // 8
    order = np.argsort(band_key, kind="stable")
    esel = esel[order]
    s_loc = src[esel] - lo
    d_loc = dst[esel] - lo
    gl = s_loc // NP
    si = s_loc % NP
    di = d_loc % NP
    assert len(esel) == E_LOC, f"core {c}: {len(esel)} edges"
    assert (np.bincount(gl, minlength=G_LOC) == EP).all()

    ea = np.asarray(edge_attr)[esel]          # [E_LOC, 16] sorted order

    # gather matrix (+ ones row for the gbm_b2 bias trick): [65, 8*512]
    gmat = np.zeros((NP + 1, E_LOC), np.float32)
    gmat[si, np.arange(E_LOC)] = 1.0
    gmat[NP, :] = 1.0
    # scatter matrix chunks: [128, 32*64]; chunk (g,ch) -> cols (g*4+ch)*64
    smat = np.zeros((128, E_LOC // 128 * NP), np.float32)
    for g in range(G_LOC):
        for ch in range(EP // 128):
            sel = slice(g * EP + ch * 128, g * EP + (ch + 1) * 128)
            blk = np.zeros((128, NP), np.float32)
            blk[np.arange(128), di[sel]] = 1.0
            smat[:, (g * 4 + ch) * NP:(g * 4 + ch + 1) * NP] = blk

    # banded (padded) edge layout for the final stage; host-built Eind
    eab = np.zeros((NB * BS, EDGE_F), np.float32)
    eib = np.zeros((NB, BS + 9, 512), np.float32)
    eib[:, BS:BS + 8, :] = (np.arange(512) // NP == np.arange(8)[:, None])
    eib[:, BS + 8, :] = 1.0
    bands = gl * 8 + si // 8
    for b in range(NB):
        sel = np.nonzero(bands == b)[0]
        nb = len(sel)
        assert nb <= BS, f"band {b} has {nb} edges > {BS}"
        eab[b * BS:b * BS + nb] = ea[sel]
        eib[b, np.arange(nb), (si[sel] % 8) * NP + di[sel]] = 1.0

    # banded ea^T + ones row (bias trick for the folded bond bias)
    eaTbe = np.concatenate(
        [eab.T, np.ones((1, NB * BS), np.float32)], 0)   # [17, 6144]

    xc = np.asarray(x)[lo:hi]                  # [512, 32]

    out = {
        "xT": _to_bf16(xc.T),                                    # [32, 512]
        "eaT": _to_bf16(ea.T),                                   # [16, 4096]
        "eaTbe": _to_bf16(eaTbe),                                # [17, 6144]
        "gmat": _to_bf16(gmat),                                  # [65, 4096]
        "smat": _to_bf16(smat),                                  # [128, 2048]
        "eib": _to_bf16(eib),                                    # [64, 105, 512]
    }
    out.update(weights)
    return out


def _prep_shared(atom_W, atom_b, bond_W, bond_b, gbm_W1, gbm_b1, gbm_W2,
                 gbm_b2, gnn_W1, gnn_b1, gnn_W2, gnn_b2, bn_gamma, bn_beta,
                 mlp_W1, mlp_b1, mlp_W2, mlp_b2):
    bW = np.asarray(bond_W, np.float32)
    bb = np.asarray(bond_b, np.float32)
    gbW1 = np.asarray(gbm_W1, np.float32)
    gbb1 = np.asarray(gbm_b1, np.float32)
    gbW2 = np.asarray(gbm_W2, np.float32)
    W1 = np.asarray(mlp_W1, np.float32)
    W3 = W1[2 * H:3 * H]

    # bond encoder folded into the per-layer edge-MLP first layers
    wu1 = np.concatenate([bW @ gbW1[l] for l in range(L)], 1)    # [16, 512]
    bt1 = np.stack([bb @ gbW1[l] + gbb1[l] for l in range(L)])   # [4, 128]
    # ... and into the final-stage dense-C projection (+ bias row)
    w3e = np.concatenate([bW @ W3, (bb @ W3)[None, :]], 0)       # [17, 128]

    # block 0: +gbm_W2[0] (layer-0 inline u2); blocks 1-3: -gbm_W2[l]
    blocks = [gbW2[0], -gbW2[1], -gbW2[2], -gbW2[3]]
    blocks += [np.asarray(gnn_W1)[l] for l in range(L)]
    blocks += [np.asarray(gnn_W2)[l] for l in range(L)]
    wsb = np.concatenate(blocks, 1)                              # [128, 1536]

    wmlp_sb = np.concatenate([W1[0:H], W1[H:2 * H]], 1)          # [128, 256]

    bcols = np.zeros((H, 21), np.float32)
    bcols[:, 0] = np.asarray(atom_b)
    bcols[:, 1:5] = bt1.T
    bcols[:, 5:9] = np.asarray(gnn_b1).T
    bcols[:, 9:13] = np.asarray(gnn_b2).T
    bcols[:, 13:17] = np.asarray(bn_gamma).T
    bcols[:, 17:21] = np.asarray(bn_beta).T

    b2rep = np.tile(np.asarray(gbm_b2), (1, G_LOC))              # [4, 1024]

    rjx = np.zeros((128, 512), np.float32)
    j_of_p = np.arange(512) % NP
    for n in range(NP):
        rjx[n, j_of_p == n] = 1.0
        rjx[NP + n, j_of_p == n] = 1.0

    b1row = np.tile(np.asarray(mlp_b1)[None, :], (1, G_LOC * 8))  # [1, 8192]

    return {
        "b1row": _to_bf16(b1row),               # [1, 8192]
        "wA": _to_bf16(atom_W),                 # [32, 128]
        "wu1": _to_bf16(wu1),                   # [16, 512]
        "w3e": _to_bf16(w3e),                   # [17, 128]
        "wsb": _to_bf16(wsb),                   # [128, 1536]
        "wmlp": _to_bf16(wmlp_sb),              # [128, 256]
        "w2c": _to_bf16(np.asarray(mlp_W2)),    # [128, 1]
        "bcols": bcols,                         # [128, 21] f32
        "b2rep": _to_bf16(b2rep),               # [4, 1024]
        "rjx": _to_bf16(rjx),                   # [128, 512]
        "ident": _to_bf16(np.eye(128)),         # [128, 128]
    }, float(np.asarray(mlp_b2)[0])


# ---------------------------------------------------------------------------
# device program
# ---------------------------------------------------------------------------

INPUT_SPECS = {
    "xT": ([IN_F, N_LOC], bf16), "eaT": ([EDGE_F, E_LOC], bf16),
    "eaTbe": ([EDGE_F + 1, NB * BS], bf16), "gmat": ([NP + 1, E_LOC], bf16),
    "smat": ([128, 32 * NP], bf16), "eib": ([NB, BS + 9, 512], bf16),
    "wA": ([IN_F, H], bf16), "wu1": ([EDGE_F, L * H], bf16),
    "w3e": ([EDGE_F + 1, H], bf16),
    "wsb": ([H, 12 * H], bf16), "wmlp": ([H, 2 * H], bf16),
    "w2c": ([H, 1], bf16), "bcols": ([H, 21], f32),
    "b2rep": ([L, G_LOC * H], bf16), "rjx": ([128, 512], bf16),
    "ident": ([128, 128], bf16), "b1row": ([1, NB * H], bf16),
}

# DMA issue queues for the constants, in issue order
_SYNC_CONSTS = ["wu1", "eaT", "bcols", "wA", "xT", "gmat", "b2rep", "rjx"]
_SCAL_CONSTS = ["wsb", "smat", "wmlp", "w2c", "ident"]
_POOL_CONSTS = ["eaTbe", "w3e"]


def build_program(mlp_b2_val):
    nc = bass.Bass(trn_type="TRN2", num_devices=N_CORES)
    dins = {n: nc.dram_tensor(n, shp, dt, kind="ExternalInput")
            for n, (shp, dt) in INPUT_SPECS.items()}
    y = nc.dram_tensor("y", [NB, 512], f32, kind="ExternalOutput")
    b2dram = dins["b2rep"]

    with _SplitDrainTC(nc) as tc:
        with tc.tile_pool(name="const", bufs=1) as cpool, \
             tc.tile_pool(name="big", bufs=1) as bigp, \
             tc.tile_pool(name="work", bufs=2) as wp, \
             tc.tile_pool(name="fin", bufs=3) as fp, \
             tc.tile_pool(name="egp", bufs=3) as egp, \
             tc.tile_pool(name="dram", bufs=1, space="DRAM") as dram, \
             tc.tile_pool(name="psA", bufs=4, space="PSUM") as psA, \
             tc.tile_pool(name="psS", bufs=2, space="PSUM") as psS, \
             tc.tile_pool(name="psO", bufs=2, space="PSUM") as psO:

            # absorb the ncfw startup barrier as early as possible: emit the
            # dummy AllReduce before anything else so its trigger leads the
            # gpsimd queue
            dmy_in = dram.tile([16, 2], f32, name="dmy_in")
            dmy_out = dram.tile([16, 2], f32, name="dmy_out", addr_space="Shared")
            nc.gpsimd.collective_compute(
                "AllReduce", ALU.add, replica_groups=[list(range(N_CORES))],
                ins=[dmy_in.opt()], outs=[dmy_out.opt()])

            # ---- load constants (three parallel DGE queues) ----
            sb = {}
            for n in _SYNC_CONSTS + _SCAL_CONSTS + _POOL_CONSTS:
                shp, dt = INPUT_SPECS[n]
                sb[n] = cpool.tile(shp, dt, name=f"c_{n}", tag=f"c_{n}")
            for n in _SYNC_CONSTS:
                nc.sync.dma_start(sb[n][:], dins[n][:])
            for n in _SCAL_CONSTS:
                nc.scalar.dma_start(sb[n][:], dins[n][:])
            for n in _POOL_CONSTS:
                nc.gpsimd.dma_start(sb[n][:], dins[n][:])
            bc = sb["bcols"]

            def bcol(i):
                return bc[:, i:i + 1]

            wsb, wmlp = sb["wsb"], sb["wmlp"]

            def wsq(i):
                return wsb[:, i * H:(i + 1) * H]

            # ---- ec rows container for all bands (final-stage C path) ----
            EC_all = bigp.tile([128, NB * H], bf16, name="EC_all", tag="EC_all")
            nc.gpsimd.dma_start(EC_all[BS + 8:BS + 9, :], dins["b1row"][:])

            # ---- u1[l] = relu(ea @ wu1[l] + bt1[l])  (bond encoder folded) ----
            u1 = [None] * L

            def emit_u1(l):
                u1[l] = wp.tile([H, E_LOC], bf16, name=f"u1_{l}", tag="u1", bufs=4)
                for j in range(E_LOC // 512):
                    pool, tg = (psA, "psA") if j % 2 == 0 else (psO, "psO")
                    ps = pool.tile([128, 512], f32, name="ps_u1", tag=tg)
                    nc.tensor.matmul(ps[:], sb["wu1"][:, l * H:(l + 1) * H],
                                     sb["eaT"][:, j * 512:(j + 1) * 512])
                    nc.scalar.activation(u1[l][:, j * 512:(j + 1) * 512], ps[:],
                                         ACTF.Relu, bias=bcol(1 + l))

            # ---- negu2[l] = -(u1[l] @ gbm_W2[l]) edge-major, for layers 1-3 ----
            negu2 = [None] * L

            def emit_negu2(l):
                negu2[l] = wp.tile([128, E_LOC], bf16, name=f"nu2_{l}",
                                   tag="nu2", bufs=3)
                for j in range(8):
                    pool, tg = (psA, "psA") if j % 2 == 0 else (psO, "psO")
                    ps = pool.tile([128, 512], f32, name="ps_nu2", tag=tg)
                    for ch in range(4):
                        e0 = j * 512 + ch * 128
                        nc.tensor.matmul(ps[:, ch * H:(ch + 1) * H],
                                         u1[l][:, e0:e0 + 128], wsq(l))
                    dst = negu2[l][:, j * 512:(j + 1) * 512]
                    if j % 2 == 0:
                        nc.scalar.activation(dst, ps[:], ACTF.Copy)
                    else:
                        nc.vector.tensor_copy(dst, ps[:])

            # ---- ec rows for all bands (h-independent; fills window 0) ----
            def emit_ec():
                for b in range(NB):
                    pse = psS.tile([BS, H], f32, name="ps_ec", tag="psS")
                    nc.tensor.matmul(pse[:], sb["eaTbe"][:, b * BS:(b + 1) * BS],
                                     sb["w3e"][:])
                    dst = EC_all[0:BS, b * H:(b + 1) * H]
                    if b % 2 == 0:
                        nc.vector.tensor_copy(dst, pse[:])
                    else:
                        nc.scalar.activation(dst, pse[:], ACTF.Copy)

            # ---- node-major h (+bias row) via one XBAR transpose DMA ----
            h_all = bigp.tile([NP + 1, G_LOC * H], bf16, name="h_all", tag="h_all")

            def update_h_all(hts, l):
                nc.sync.dma_start(h_all[NP:NP + 1, :], b2dram[l:l + 1, :])
                for g in range(G_LOC):
                    pst = psS.tile([NP, H], f32, name="ps_ht", tag="psS")
                    nc.tensor.matmul(pst[:], hts[:, g * NP:(g + 1) * NP], sb["ident"][:])
                    dst = h_all[0:NP, g * H:(g + 1) * H]
                    if g % 2 == 0:
                        nc.scalar.activation(dst, pst[:], ACTF.Copy)
                    else:
                        nc.vector.tensor_copy(dst, pst[:])

            # ---- eib group prefetch (batched, Pool DGE queue) ----
            eg = {}

            def prefetch_eib(grp):
                t = egp.tile([BS + 9, 8, 512], bf16, name="egrp", tag="egrp",
                             bufs=3)
                src = dins["eib"][grp * 8:(grp + 1) * 8, 0:BS + 9, :]
                nc.gpsimd.dma_start(t[:, :, :], src.rearrange("i p c -> p i c"))
                eg[grp] = t

            # ---- atom encoder ----
            hT = [None] * (L + 1)
            emit_u1(0)
            hT[0] = wp.tile([H, N_LOC], bf16, name="hT0", tag="hT", bufs=3)
            ps = psA.tile([128, 512], f32, name="ps_h0", tag="psA")
            nc.tensor.matmul(ps[:], sb["wA"][:], sb["xT"][:])
            nc.vector.tensor_scalar_add(hT[0][:], ps[:], bcol(0))
            update_h_all(hT[0], 0)

            # ---- GINE layers ----
            for l in range(L):
                # messages, edge-major
                m_sb = wp.tile([128, E_LOC], bf16, name=f"m_{l}", tag="m")
                for g in range(G_LOC):
                    pool, tg = (psA, "psA") if g % 2 == 0 else (psO, "psO")
                    psm = pool.tile([128, 512], f32, name="ps_m", tag=tg)
                    if l == 0:
                        for ch in range(4):
                            e0 = g * EP + ch * 128
                            nc.tensor.matmul(psm[:, ch * H:(ch + 1) * H],
                                             u1[0][:, e0:e0 + 128],
                                             wsq(0), start=True, stop=False)
                            nc.tensor.matmul(psm[:, ch * H:(ch + 1) * H],
                                             sb["gmat"][0:NP + 1, e0:e0 + 128],
                                             h_all[0:NP + 1, g * H:(g + 1) * H],
                                             start=False, stop=True)
                        for ch in range(4):
                            dst = m_sb[:, (g * 4 + ch) * H:(g * 4 + ch + 1) * H]
                            src = psm[:, ch * H:(ch + 1) * H]
                            if ch % 2 == 0:
                                nc.scalar.activation(dst, src, ACTF.Relu)
                            else:
                                nc.vector.tensor_scalar_max(dst, src, 0.0)
                    else:
                        for ch in range(4):
                            e0 = g * EP + ch * 128
                            nc.tensor.matmul(psm[:, ch * H:(ch + 1) * H],
                                             sb["gmat"][0:NP + 1, e0:e0 + 128],
                                             h_all[0:NP + 1, g * H:(g + 1) * H])
                        # m = relu(gather + u2) = max(gather, -u2) - (-u2)
                        nsl = negu2[l][:, g * 512:(g + 1) * 512]
                        tmp = wp.tile([128, 512], bf16, name="mtmp", tag="mtmp",
                                      bufs=4)
                        nc.vector.tensor_tensor(tmp[:], psm[:], nsl, ALU.max)
                        nc.gpsimd.tensor_tensor(m_sb[:, g * 512:(g + 1) * 512],
                                                tmp[:], nsl, ALU.subtract)

                # scatter-aggregate (feature-major out)
                psagg = psA.tile([128, 512], f32, name="ps_agg", tag="psA")
                for g in range(G_LOC):
                    for ch in range(4):
                        nc.tensor.matmul(psagg[:, g * NP:(g + 1) * NP],
                                         m_sb[:, (g * 4 + ch) * H:(g * 4 + ch + 1) * H],
                                         sb["smat"][:, (g * 4 + ch) * NP:(g * 4 + ch + 1) * NP],
                                         start=(ch == 0), stop=(ch == 3))
                zT = wp.tile([H, N_LOC], bf16, name=f"zT_{l}", tag="zT")
                nc.vector.tensor_tensor(zT[:], hT[l][:], psagg[:], ALU.add)

                # node MLP
                ps1 = psA.tile([128, 512], f32, name="ps_z1", tag="psA")
                nc.tensor.matmul(ps1[:], wsq(4 + l), zT[:])
                y1T = wp.tile([H, N_LOC], bf16, name=f"y1_{l}", tag="y1")
                nc.scalar.activation(y1T[:], ps1[:], ACTF.Relu, bias=bcol(5 + l))
                ps2 = psA.tile([128, 512], f32, name="ps_z2", tag="psA")
                nc.tensor.matmul(ps2[:], wsq(8 + l), y1T[:])
                z2T = wp.tile([H, N_LOC], f32, name=f"z2_{l}", tag="z2")
                nc.vector.tensor_scalar_add(z2T[:], ps2[:], bcol(9 + l))

                # batchnorm stats + AllReduce
                st = wp.tile([H, 2], f32, name=f"st_{l}", tag="st")
                nc.vector.reduce_sum(st[:, 0:1], z2T[:], axis=AX.X)
                sq = wp.tile([H, N_LOC], f32, name=f"sq_{l}", tag="sq")
                nc.vector.tensor_tensor(sq[:], z2T[:], z2T[:], ALU.mult)
                nc.vector.reduce_sum(st[:, 1:2], sq[:], axis=AX.X)
                cc_in = dram.tile([H, 2], f32, name=f"ccin_{l}")
                cc_out = dram.tile([H, 2], f32, name=f"ccout_{l}", addr_space="Shared")
                nc.sync.dma_start(cc_in[:], st[:])
                nc.gpsimd.collective_compute(
                    "AllReduce", ALU.add,
                    replica_groups=[list(range(N_CORES))],
                    ins=[cc_in.opt()], outs=[cc_out.opt()])

                # fill the collective window with h-independent work
                if l == 0:
                    emit_u1(1)
                    emit_negu2(1)
                    emit_u1(2)
                    emit_negu2(2)
                    emit_u1(3)
                    emit_negu2(3)
                    emit_ec()
                elif l == 1:
                    prefetch_eib(0)
                elif l == 2:
                    prefetch_eib(1)

                st2 = wp.tile([H, 2], f32, name=f"st2_{l}", tag="st2")
                nc.sync.dma_start(st2[:], cc_out[:])

                # alpha = gamma*rsqrt(var+eps); beta' = beta - mu*alpha
                s_ = wp.tile([H, 6], f32, name=f"bn_{l}", tag="bn")
                nc.vector.tensor_scalar_mul(s_[:, 0:1], st2[:, 0:1], -1.0 / NTOT)   # -mu
                nc.vector.tensor_scalar(s_[:, 1:2], s_[:, 0:1], st2[:, 0:1],
                                        BN_EPS * NTOT, ALU.mult, ALU.add)  # -S^2/N + N*eps
                nc.vector.tensor_tensor(s_[:, 2:3], s_[:, 1:2], st2[:, 1:2], ALU.add)  # +Q
                nc.scalar.activation(s_[:, 3:4], s_[:, 2:3], ACTF.Sqrt,
                                     scale=1.0 / NTOT)                 # sqrt(var+eps)
                nc.vector.reciprocal(s_[:, 5:6], s_[:, 3:4])
                nc.vector.tensor_tensor(s_[:, 4:5], s_[:, 5:6], bcol(13 + l), ALU.mult)  # alpha
                bnb = wp.tile([H, 1], f32, name=f"bnb_{l}", tag="bnb")
                nc.vector.tensor_scalar(bnb[:], s_[:, 4:5], s_[:, 0:1],
                                        bcol(17 + l), ALU.mult, ALU.add)     # beta' = alpha*(-mu)+beta

                hT[l + 1] = wp.tile([H, N_LOC], bf16, name=f"hT{l + 1}", tag="hT", bufs=3)
                nc.scalar.activation(hT[l + 1][:], z2T[:], ACTF.Relu,
                                     bias=bnb[:], scale=s_[:, 4:5])
                if l + 1 < L:
                    update_h_all(hT[l + 1], l + 1)

            # ---- final stage ----
            h4 = hT[L]
            # B node-major
            Bn = bigp.tile([128, 512], bf16, name="Bn", tag="Bn")
            for ch in range(4):
                psb = psS.tile([128, H], f32, name="ps_bn", tag="psS")
                nc.tensor.matmul(psb[:], h4[:, ch * H:(ch + 1) * H], wmlp[:, H:2 * H])
                nc.scalar.activation(Bn[:, ch * H:(ch + 1) * H], psb[:], ACTF.Copy)

            for b8 in range(NB // 8):
                g = b8
                if b8 + 2 < NB // 8:
                    prefetch_eib(b8 + 2)
                # phase A: A' rows into EC_all rows BS..BS+8 (PE placement)
                for i in range(8):
                    b = b8 * 8 + i
                    psa = psS.tile([128, H], f32, name="ps_a8", tag="psS")
                    nc.tensor.matmul(psa[BS:BS + 8, :],
                                     h4[:, g * NP + i * 8:g * NP + i * 8 + 8],
                                     wmlp[:, 0:H], tile_position=(0, 96))
                    nc.vector.tensor_copy(EC_all[BS:BS + 8, b * H:(b + 1) * H],
                                          psa[BS:BS + 8, :])
                # phase B: dense matmul stream + relus
                rts = []
                for i in range(8):
                    b = b8 * 8 + i
                    pt = psA.tile([128, 512], f32, name="ps_T", tag="psA")
                    rb = (g % 2) * NP
                    nc.tensor.matmul(pt[:], Bn[rb:rb + NP, (g // 2) * H:(g // 2 + 1) * H],
                                     sb["rjx"][rb:rb + NP, :], start=True, stop=False)
                    nc.tensor.matmul(pt[:], EC_all[0:BS + 9, b * H:(b + 1) * H],
                                     eg[b8][:, i, :], start=False, stop=True)
                    relu_t = fp.tile([128, 512], bf16, name="relu_t", tag="relu_t",
                                     bufs=9)
                    if b % 2 == 0:
                        nc.scalar.activation(relu_t[:], pt[:], ACTF.Relu)
                    else:
                        nc.vector.tensor_scalar_max(relu_t[:], pt[:], 0.0)
                    rts.append(relu_t)
                # phase C: w2 reductions (col-tiled groups of 4) + output
                for half in range(2):
                    po = psO.tile([128, 512], f32, name="ps_o", tag="psO")
                    for i in range(4):
                        nc.tensor.matmul(po[32 * i:32 * i + 1, :], sb["w2c"][:],
                                         rts[half * 4 + i][:],
                                         tile_position=(0, 32 * i))
                    stage = fp.tile([128, 512], f32, name="ostage", tag="ostage")
                    nc.scalar.activation(stage[:], po[:], ACTF.Copy,
                                         bias=mlp_b2_val)
                    nc.sync.dma_start(y[b8 * 8 + half * 4:b8 * 8 + half * 4 + 4, :],
                                      stage[0:97:32, :])

    _split_multi_waits(nc)
    return nc


def _split_multi_waits(nc, cap=1):
    """This walrus build accepts at most one sync wait per instruction; move
    extra waits onto same-engine NoOps inserted immediately before."""
    for fn in nc.m.functions:
        for bb in fn.blocks:
            out = []
            for inst in bb.instructions:
                si = inst.sync_info
                waits = list(si.on_wait) if si and si.on_wait else []
                if len(waits) > cap:
                    for w in waits[:-cap]:
                        nop = mybir.InstNoOp(
                            name=nc.get_next_instruction_name(),
                            sync_info=mybir.SyncInfo(on_wait=[w], on_update=[]),
                            bass_nofuse=True,
                            engine=inst.engine,
                        )
                        out.append(nop)
                    si.on_wait = waits[-cap:]
                out.append(inst)
            bb.instructions = out


# ---------------------------------------------------------------------------
# entry point
# ---------------------------------------------------------------------------

def kernel(**inputs):
    x = np.asarray(inputs["x"])
    edge_attr = np.asarray(inputs["edge_attr"])
    ei = np.asarray(inputs["edge_index"])
    src, dst = ei[0], ei[1]

    shared, b2val = _prep_shared(
        inputs["atom_W"], inputs["atom_b"], inputs["bond_W"], inputs["bond_b"],
        inputs["gbm_W1"], inputs["gbm_b1"], inputs["gbm_W2"], inputs["gbm_b2"],
        inputs["gnn_W1"], inputs["gnn_b1"], inputs["gnn_W2"], inputs["gnn_b2"],
        inputs["bn_gamma"], inputs["bn_beta"], inputs["mlp_W1"], inputs["mlp_b1"],
        inputs["mlp_W2"], inputs["mlp_b2"])

    in_maps = []
    for c in range(N_CORES):
        m = _prep_core(c, x, edge_attr, src, dst, shared)
        in_maps.append({k: np.ascontiguousarray(v) for k, v in m.items()})

    nc = build_program(b2val)
    trace = bool(int(os.environ.get("KERNEL_TRACE", "0")))
    res = run_bass_kernel_spmd(nc, in_maps, list(range(N_CORES)), trace=trace)
    kernel.last_exec_time_ns = res.exec_time_ns
    kernel.last_trace = res.instructions_and_trace

    out = np.concatenate([res.results[c]["y"].reshape(-1) for c in range(N_CORES)])
    return out.reshape(G * NP * NP, 1).astype(np.float32)


kernel.last_exec_time_ns = None
kernel.last_trace = None


# revision 16
# speedup vs baseline: 1.0954x; 1.0954x over previous
"""Trainium2 Bass kernel for nn_LinearEmbed (GINE message passing + all-pairs edge embed).

Sharding: data-parallel over graphs. 64 graphs -> 8 cores x 8 graphs.
Cross-core coupling: batchnorm statistics (AllReduce of [128,2] per layer).

Layout conventions (per core, G_loc=8 graphs, 512 nodes, 4096 edges):
  feature-major: [H=128 partitions, rows free]  (hT, u1T, ...)
  edge-major:    [128 edge partitions, H free]  (messages m, negu2, ec)
All matmuls in bf16 (f32 PSUM accumulate).

v2 notes:
  - bond encoder folded into u1 / ec weights on host (wu1 = bond_W @ gbm_W1,
    w3e = bond_W @ mlp_W1[2H:3H] + bias row): eT / eTb never materialized.
  - all u1 layers + negated-u2 (edge MLP part of messages) + ec emitted
    inside layer-0's collective window; layers 1-3 then only run the
    h-dependent gather matmuls and a 2-op relu  m = max(g, -u2) - (-u2).
  - const DMAs split across the SP / Activation / Pool DGE queues.
  - eib selector DMAs batched per 8-band group on the Pool queue.
  - per-band A' transposes replaced by one node-major A' (4 matmuls) +
    cross-partition vector copies.
"""

import os
import numpy as np
import ml_dtypes

import concourse.bass as bass
import concourse.mybir as mybir
import concourse.tile as tile
from concourse.vector_clock import ScopedClock
from concourse.bass_utils import run_bass_kernel_spmd

# problem constants
G, NP, EP, H = 64, 64, 512, 128
IN_F, EDGE_F, L = 32, 16, 4
BN_EPS = 1e-5
N_CORES = 8
G_LOC = G // N_CORES          # 8 graphs per core
N_LOC = G_LOC * NP            # 512 nodes
E_LOC = G_LOC * EP            # 4096 edges
NB = G_LOC * 8                # 64 bands per core (8 i-bands per graph)
BS = 96                       # band slot budget (max edges per band)
NTOT = float(G * NP)          # batchnorm population

f32 = mybir.dt.float32
bf16 = mybir.dt.bfloat16
AX = mybir.AxisListType
ALU = mybir.AluOpType
ACTF = mybir.ActivationFunctionType

bf = ml_dtypes.bfloat16


def _to_bf16(a):
    return np.asarray(a, dtype=np.float32).astype(bf)


class _SplitDrainTC(tile.TileContext):
    """Tail drain in this walrus build accepts only one sync wait; split the
    global-clock waits across multiple drain instructions."""

    def _drain_and_barrier(self, tick_clock, wait_clock):
        drain_inst = self.nc.sync.drain()
        wait_clock.add_sem_waits(
            drain_inst.ins, ScopedClock({None: tick_clock.global_clock})
        )
        si = drain_inst.ins.sync_info
        waits = list(si.on_wait or [])
        if len(waits) > 1:
            si.on_wait = [waits[0]]
            for w in waits[1:]:
                extra = self.nc.sync.drain()
                extra.ins.sync_info = mybir.SyncInfo(on_wait=[w], on_update=[])
        self.nc.all_engine_barrier()
        assert self.sems is not None
        popped = self.nc._tile_sem_poison_stack.pop()
        assert popped is self._sem_poison
        self.nc.clear_and_free_semaphores(list(self.sems.allocated().values()))


# ---------------------------------------------------------------------------
# host-side preprocessing: shard + sort + one ndarray per SBUF constant
# ---------------------------------------------------------------------------

def _prep_core(c, x, edge_attr, src, dst, weights):
    g0 = c * G_LOC
    lo, hi = g0 * NP, (g0 + G_LOC) * NP
    mask = (src >= lo) & (src < hi)
    esel = np.nonzero(mask)[0]
    assert ((dst[esel] >= lo) & (dst[esel] < hi)).all(), "cross-shard edge"

    # stable sort local edges by (graph, band)
    s_loc = src[esel] - lo
    band_key = (s_loc // NP) * 8 + (s_loc % NP) // 8
    order = np.argsort(band_key, kind="stable")
    esel = esel[order]
    s_loc = src[esel] - lo
    d_loc = dst[esel] - lo
    gl = s_loc // NP
    si = s_loc % NP
    di = d_loc % NP
    assert len(esel) == E_LOC, f"core {c}: {len(esel)} edges"
    assert (np.bincount(gl, minlength=G_LOC) == EP).all()

    ea = np.asarray(edge_attr)[esel]          # [E_LOC, 16] sorted order

    # gather matrix (+ ones row for the gbm_b2 bias trick): [65, 8*512]
    gmat = np.zeros((NP + 1, E_LOC), np.float32)
    gmat[si, np.arange(E_LOC)] = 1.0
    gmat[NP, :] = 1.0
    # scatter matrix chunks: [128, 32*64]; chunk (g,ch) -> cols (g*4+ch)*64
    smat = np.zeros((128, E_LOC // 128 * NP), np.float32)
    for g in range(G_LOC):
        for ch in range(EP // 128):
            sel = slice(g * EP + ch * 128, g * EP + (ch + 1) * 128)
            blk = np.zeros((128, NP), np.float32)
            blk[np.arange(128), di[sel]] = 1.0
            smat[:, (g * 4 + ch) * NP:(g * 4 + ch + 1) * NP] = blk

    # banded (padded) edge layout for the final stage; host-built Eind
    eab = np.zeros((NB * BS, EDGE_F), np.float32)
    eib = np.zeros((NB, BS + 9, 512), np.float32)
    eib[:, BS:BS + 8, :] = (np.arange(512) // NP == np.arange(8)[:, None])
    eib[:, BS + 8, :] = 1.0
    bands = gl * 8 + si // 8
    for b in range(NB):
        sel = np.nonzero(bands == b)[0]
        nb = len(sel)
        assert nb <= BS, f"band {b} has {nb} edges > {BS}"
        eab[b * BS:b * BS + nb] = ea[sel]
        eib[b, np.arange(nb), (si[sel] % 8) * NP + di[sel]] = 1.0

    # banded ea^T + ones row (bias trick for the folded bond bias)
    eaTbe = np.concatenate(
        [eab.T, np.ones((1, NB * BS), np.float32)], 0)   # [17, 6144]

    xc = np.asarray(x)[lo:hi]                  # [512, 32]

    out = {
        "xT": _to_bf16(xc.T),                                    # [32, 512]
        "eaT": _to_bf16(ea.T),                                   # [16, 4096]
        "eaTbe": _to_bf16(eaTbe),                                # [17, 6144]
        "gmat": _to_bf16(gmat),                                  # [65, 4096]
        "smat": _to_bf16(smat),                                  # [128, 2048]
        "eib": _to_bf16(eib),                                    # [64, 105, 512]
    }
    out.update(weights)
    return out


def _prep_shared(atom_W, atom_b, bond_W, bond_b, gbm_W1, gbm_b1, gbm_W2,
                 gbm_b2, gnn_W1, gnn_b1, gnn_W2, gnn_b2, bn_gamma, bn_beta,
                 mlp_W1, mlp_b1, mlp_W2, mlp_b2):
    bW = np.asarray(bond_W, np.float32)
    bb = np.asarray(bond_b, np.float32)
    gbW1 = np.asarray(gbm_W1, np.float32)
    gbb1 = np.asarray(gbm_b1, np.float32)
    gbW2 = np.asarray(gbm_W2, np.float32)
    W1 = np.asarray(mlp_W1, np.float32)
    W3 = W1[2 * H:3 * H]

    # bond encoder folded into the per-layer edge-MLP first layers
    wu1 = np.concatenate([bW @ gbW1[l] for l in range(L)], 1)    # [16, 512]
    bt1 = np.stack([bb @ gbW1[l] + gbb1[l] for l in range(L)])   # [4, 128]
    # ... and into the final-stage dense-C projection (+ bias row)
    w3e = np.concatenate([bW @ W3, (bb @ W3)[None, :]], 0)       # [17, 128]

    # block 0: +gbm_W2[0] (layer-0 inline u2); blocks 1-3: -gbm_W2[l]
    blocks = [gbW2[0], -gbW2[1], -gbW2[2], -gbW2[3]]
    blocks += [np.asarray(gnn_W1)[l] for l in range(L)]
    blocks += [np.asarray(gnn_W2)[l] for l in range(L)]
    wsb = np.concatenate(blocks, 1)                              # [128, 1536]

    wmlp_sb = np.concatenate([W1[0:H], W1[H:2 * H]], 1)          # [128, 256]

    bcols = np.zeros((H, 21), np.float32)
    bcols[:, 0] = np.asarray(atom_b)
    bcols[:, 1:5] = bt1.T
    bcols[:, 5:9] = np.asarray(gnn_b1).T
    bcols[:, 9:13] = np.asarray(gnn_b2).T
    bcols[:, 13:17] = np.asarray(bn_gamma).T
    bcols[:, 17:21] = np.asarray(bn_beta).T

    b2rep = np.tile(np.asarray(gbm_b2), (1, G_LOC))              # [4, 1024]

    rjx = np.zeros((128, 512), np.float32)
    j_of_p = np.arange(512) % NP
    for n in range(NP):
        rjx[n, j_of_p == n] = 1.0
        rjx[NP + n, j_of_p == n] = 1.0

    b1row = np.tile(np.asarray(mlp_b1)[None, :], (1, G_LOC * 8))  # [1, 8192]

    return {
        "b1row": _to_bf16(b1row),               # [1, 8192]
        "wA": _to_bf16(atom_W),                 # [32, 128]
        "wu1": _to_bf16(wu1),                   # [16, 512]
        "w3e": _to_bf16(w3e),                   # [17, 128]
        "wsb": _to_bf16(wsb),                   # [128, 1536]
        "wmlp": _to_bf16(wmlp_sb),              # [128, 256]
        "w2c": _to_bf16(np.asarray(mlp_W2)),    # [128, 1]
        "bcols": bcols,                         # [128, 21] f32
        "b2rep": _to_bf16(b2rep),               # [4, 1024]
        "rjx": _to_bf16(rjx),                   # [128, 512]
        "ident": _to_bf16(np.eye(128)),         # [128, 128]
    }, float(np.asarray(mlp_b2)[0])


# ---------------------------------------------------------------------------
# device program
# ---------------------------------------------------------------------------

INPUT_SPECS = {
    "xT": ([IN_F, N_LOC], bf16), "eaT": ([EDGE_F, E_LOC], bf16),
    "eaTbe": ([EDGE_F + 1, NB * BS], bf16), "gmat": ([NP + 1, E_LOC], bf16),
    "smat": ([128, 32 * NP], bf16), "eib": ([NB, BS + 9, 512], bf16),
    "wA": ([IN_F, H], bf16), "wu1": ([EDGE_F, L * H], bf16),
    "w3e": ([EDGE_F + 1, H], bf16),
    "wsb": ([H, 12 * H], bf16), "wmlp": ([H, 2 * H], bf16),
    "w2c": ([H, 1], bf16), "bcols": ([H, 21], f32),
    "b2rep": ([L, G_LOC * H], bf16), "rjx": ([128, 512], bf16),
    "ident": ([128, 128], bf16), "b1row": ([1, NB * H], bf16),
}

# DMA issue queues for the constants, in issue order
_SYNC_CONSTS = ["eaT", "gmat", "xT", "b2rep", "rjx"]
_SCAL_CONSTS = ["wu1", "bcols", "wA", "wsb", "smat", "wmlp", "w2c", "ident"]
_POOL_CONSTS = ["eaTbe", "w3e"]


def build_program(mlp_b2_val):
    nc = bass.Bass(trn_type="TRN2", num_devices=N_CORES)
    dins = {n: nc.dram_tensor(n, shp, dt, kind="ExternalInput")
            for n, (shp, dt) in INPUT_SPECS.items()}
    y = nc.dram_tensor("y", [NB, 512], f32, kind="ExternalOutput")
    b2dram = dins["b2rep"]

    with _SplitDrainTC(nc) as tc:
        with tc.tile_pool(name="const", bufs=1) as cpool, \
             tc.tile_pool(name="big", bufs=1) as bigp, \
             tc.tile_pool(name="work", bufs=2) as wp, \
             tc.tile_pool(name="fin", bufs=3) as fp, \
             tc.tile_pool(name="egp", bufs=3) as egp, \
             tc.tile_pool(name="dram", bufs=1, space="DRAM") as dram, \
             tc.tile_pool(name="psA", bufs=4, space="PSUM") as psA, \
             tc.tile_pool(name="psS", bufs=2, space="PSUM") as psS, \
             tc.tile_pool(name="psO", bufs=2, space="PSUM") as psO:

            # absorb the ncfw startup barrier as early as possible: emit the
            # dummy AllReduce before anything else so its trigger leads the
            # gpsimd queue
            dmy_in = dram.tile([16, 2], f32, name="dmy_in")
            dmy_out = dram.tile([16, 2], f32, name="dmy_out", addr_space="Shared")
            nc.gpsimd.collective_compute(
                "AllReduce", ALU.add, replica_groups=[list(range(N_CORES))],
                ins=[dmy_in.opt()], outs=[dmy_out.opt()])

            # ---- load constants (three parallel DGE queues) ----
            sb = {}
            for n in _SYNC_CONSTS + _SCAL_CONSTS + _POOL_CONSTS:
                shp, dt = INPUT_SPECS[n]
                sb[n] = cpool.tile(shp, dt, name=f"c_{n}", tag=f"c_{n}")
            for n in _SYNC_CONSTS:
                nc.sync.dma_start(sb[n][:], dins[n][:])
            for n in _SCAL_CONSTS:
                nc.scalar.dma_start(sb[n][:], dins[n][:])
            for n in _POOL_CONSTS:
                nc.gpsimd.dma_start(sb[n][:], dins[n][:])
            bc = sb["bcols"]

            def bcol(i):
                return bc[:, i:i + 1]

            wsb, wmlp = sb["wsb"], sb["wmlp"]

            def wsq(i):
                return wsb[:, i * H:(i + 1) * H]

            # ---- ec rows container for all bands (final-stage C path) ----
            EC_all = bigp.tile([128, NB * H], bf16, name="EC_all", tag="EC_all")
            nc.gpsimd.dma_start(EC_all[BS + 8:BS + 9, :], dins["b1row"][:])

            # ---- u1[l] = relu(ea @ wu1[l] + bt1[l])  (bond encoder folded) ----
            u1 = [None] * L

            def emit_u1(l):
                u1[l] = wp.tile([H, E_LOC], bf16, name=f"u1_{l}", tag="u1", bufs=4)
                for j in range(E_LOC // 512):
                    pool, tg = (psA, "psA") if j % 2 == 0 else (psO, "psO")
                    ps = pool.tile([128, 512], f32, name="ps_u1", tag=tg)
                    nc.tensor.matmul(ps[:], sb["wu1"][:, l * H:(l + 1) * H],
                                     sb["eaT"][:, j * 512:(j + 1) * 512])
                    nc.scalar.activation(u1[l][:, j * 512:(j + 1) * 512], ps[:],
                                         ACTF.Relu, bias=bcol(1 + l))

            # ---- negu2[l] = -(u1[l] @ gbm_W2[l]) edge-major, for layers 1-3 ----
            negu2 = [None] * L

            def emit_negu2(l):
                negu2[l] = wp.tile([128, E_LOC], bf16, name=f"nu2_{l}",
                                   tag="nu2", bufs=3)
                for j in range(8):
                    pool, tg = (psA, "psA") if j % 2 == 0 else (psO, "psO")
                    ps = pool.tile([128, 512], f32, name="ps_nu2", tag=tg)
                    for ch in range(4):
                        e0 = j * 512 + ch * 128
                        nc.tensor.matmul(ps[:, ch * H:(ch + 1) * H],
                                         u1[l][:, e0:e0 + 128], wsq(l))
                    dst = negu2[l][:, j * 512:(j + 1) * 512]
                    if j % 2 == 0:
                        nc.scalar.activation(dst, ps[:], ACTF.Copy)
                    else:
                        nc.vector.tensor_copy(dst, ps[:])

            # ---- ec rows for all bands (h-independent; fills window 0) ----
            def emit_ec():
                for b in range(NB):
                    pse = psS.tile([BS, H], f32, name="ps_ec", tag="psS")
                    nc.tensor.matmul(pse[:], sb["eaTbe"][:, b * BS:(b + 1) * BS],
                                     sb["w3e"][:])
                    dst = EC_all[0:BS, b * H:(b + 1) * H]
                    if b % 2 == 0:
                        nc.vector.tensor_copy(dst, pse[:])
                    else:
                        nc.scalar.activation(dst, pse[:], ACTF.Copy)

            # ---- node-major h (+bias row) via one XBAR transpose DMA ----
            h_all = bigp.tile([NP + 1, G_LOC * H], bf16, name="h_all", tag="h_all")

            def update_h_all(hts, l):
                nc.sync.dma_start(h_all[NP:NP + 1, :], b2dram[l:l + 1, :])
                for g in range(G_LOC):
                    pst = psS.tile([NP, H], f32, name="ps_ht", tag="psS")
                    nc.tensor.matmul(pst[:], hts[:, g * NP:(g + 1) * NP], sb["ident"][:])
                    dst = h_all[0:NP, g * H:(g + 1) * H]
                    if g % 2 == 0:
                        nc.scalar.activation(dst, pst[:], ACTF.Copy)
                    else:
                        nc.vector.tensor_copy(dst, pst[:])

            # ---- eib prefetch: pairs of bands, alternating DGE queues ----
            eg = {}

            def prefetch_eib(pair):
                t = egp.tile([BS + 9, 2, 512], bf16, name="egrp", tag="egrp",
                             bufs=10)
                src = dins["eib"][pair * 2:(pair + 1) * 2, 0:BS + 9, :]
                eng = nc.sync if pair % 2 == 0 else nc.scalar
                eng.dma_start(t[:, :, :], src.rearrange("i p c -> p i c"))
                eg[pair] = t

            # ---- atom encoder ----
            hT = [None] * (L + 1)
            emit_u1(0)
            hT[0] = wp.tile([H, N_LOC], bf16, name="hT0", tag="hT", bufs=3)
            ps = psA.tile([128, 512], f32, name="ps_h0", tag="psA")
            nc.tensor.matmul(ps[:], sb["wA"][:], sb["xT"][:])
            nc.vector.tensor_scalar_add(hT[0][:], ps[:], bcol(0))
            update_h_all(hT[0], 0)

            # ---- GINE layers ----
            for l in range(L):
                # messages, edge-major
                m_sb = wp.tile([128, E_LOC], bf16, name=f"m_{l}", tag="m")
                for g in range(G_LOC):
                    pool, tg = (psA, "psA") if g % 2 == 0 else (psO, "psO")
                    psm = pool.tile([128, 512], f32, name="ps_m", tag=tg)
                    if l == 0:
                        for ch in range(4):
                            e0 = g * EP + ch * 128
                            nc.tensor.matmul(psm[:, ch * H:(ch + 1) * H],
                                             u1[0][:, e0:e0 + 128],
                                             wsq(0), start=True, stop=False)
                            nc.tensor.matmul(psm[:, ch * H:(ch + 1) * H],
                                             sb["gmat"][0:NP + 1, e0:e0 + 128],
                                             h_all[0:NP + 1, g * H:(g + 1) * H],
                                             start=False, stop=True)
                        for ch in range(4):
                            dst = m_sb[:, (g * 4 + ch) * H:(g * 4 + ch + 1) * H]
                            src = psm[:, ch * H:(ch + 1) * H]
                            if ch % 2 == 0:
                                nc.scalar.activation(dst, src, ACTF.Relu)
                            else:
                                nc.vector.tensor_scalar_max(dst, src, 0.0)
                    else:
                        for ch in range(4):
                            e0 = g * EP + ch * 128
                            nc.tensor.matmul(psm[:, ch * H:(ch + 1) * H],
                                             sb["gmat"][0:NP + 1, e0:e0 + 128],
                                             h_all[0:NP + 1, g * H:(g + 1) * H])
                        # m = relu(gather + u2) = max(gather, -u2) - (-u2)
                        nsl = negu2[l][:, g * 512:(g + 1) * 512]
                        tmp = wp.tile([128, 512], bf16, name="mtmp", tag="mtmp",
                                      bufs=4)
                        nc.vector.tensor_tensor(tmp[:], psm[:], nsl, ALU.max)
                        nc.gpsimd.tensor_tensor(m_sb[:, g * 512:(g + 1) * 512],
                                                tmp[:], nsl, ALU.subtract)

                # scatter-aggregate (feature-major out)
                psagg = psA.tile([128, 512], f32, name="ps_agg", tag="psA")
                for g in range(G_LOC):
                    for ch in range(4):
                        nc.tensor.matmul(psagg[:, g * NP:(g + 1) * NP],
                                         m_sb[:, (g * 4 + ch) * H:(g * 4 + ch + 1) * H],
                                         sb["smat"][:, (g * 4 + ch) * NP:(g * 4 + ch + 1) * NP],
                                         start=(ch == 0), stop=(ch == 3))
                zT = wp.tile([H, N_LOC], bf16, name=f"zT_{l}", tag="zT")
                nc.vector.tensor_tensor(zT[:], hT[l][:], psagg[:], ALU.add)

                # node MLP
                ps1 = psA.tile([128, 512], f32, name="ps_z1", tag="psA")
                nc.tensor.matmul(ps1[:], wsq(4 + l), zT[:])
                y1T = wp.tile([H, N_LOC], bf16, name=f"y1_{l}", tag="y1")
                nc.scalar.activation(y1T[:], ps1[:], ACTF.Relu, bias=bcol(5 + l))
                ps2 = psA.tile([128, 512], f32, name="ps_z2", tag="psA")
                nc.tensor.matmul(ps2[:], wsq(8 + l), y1T[:])
                z2T = wp.tile([H, N_LOC], f32, name=f"z2_{l}", tag="z2")
                nc.scalar.activation(z2T[:], ps2[:], ACTF.Identity, bias=bcol(9 + l))

                # batchnorm stats: S on vector, Q on scalar (Square + accum)
                st = wp.tile([H, 2], f32, name=f"st_{l}", tag="st")
                nc.vector.reduce_sum(st[:, 0:1], z2T[:], axis=AX.X)
                sq = wp.tile([H, N_LOC], f32, name=f"sq_{l}", tag="sq")
                nc.scalar.activation(sq[:], z2T[:], ACTF.Square,
                                     accum_out=st[:, 1:2])
                cc_in = dram.tile([H, 2], f32, name=f"ccin_{l}")
                cc_out = dram.tile([H, 2], f32, name=f"ccout_{l}", addr_space="Shared")
                nc.sync.dma_start(cc_in[:], st[:])
                nc.gpsimd.collective_compute(
                    "AllReduce", ALU.add,
                    replica_groups=[list(range(N_CORES))],
                    ins=[cc_in.opt()], outs=[cc_out.opt()])

                # fill the collective window with h-independent work
                if l == 0:
                    emit_u1(1)
                    emit_negu2(1)
                    emit_ec()
                elif l == 1:
                    emit_u1(2)
                    emit_negu2(2)
                elif l == 2:
                    emit_u1(3)
                    emit_negu2(3)
                    prefetch_eib(0)
                    prefetch_eib(1)
                elif l == 3:
                    prefetch_eib(2)
                    prefetch_eib(3)

                st2 = wp.tile([H, 2], f32, name=f"st2_{l}", tag="st2")
                nc.sync.dma_start(st2[:], cc_out[:])

                # alpha = gamma*rsqrt(var+eps); beta' = beta - mu*alpha
                s_ = wp.tile([H, 6], f32, name=f"bn_{l}", tag="bn")
                nc.vector.tensor_scalar_mul(s_[:, 0:1], st2[:, 0:1], -1.0 / NTOT)   # -mu
                nc.vector.tensor_scalar(s_[:, 1:2], s_[:, 0:1], st2[:, 0:1],
                                        BN_EPS * NTOT, ALU.mult, ALU.add)  # -S^2/N + N*eps
                nc.vector.tensor_tensor(s_[:, 2:3], s_[:, 1:2], st2[:, 1:2], ALU.add)  # +Q
                nc.scalar.activation(s_[:, 3:4], s_[:, 2:3], ACTF.Sqrt,
                                     scale=1.0 / NTOT)                 # sqrt(var+eps)
                nc.vector.reciprocal(s_[:, 5:6], s_[:, 3:4])
                nc.vector.tensor_tensor(s_[:, 4:5], s_[:, 5:6], bcol(13 + l), ALU.mult)  # alpha
                bnb = wp.tile([H, 1], f32, name=f"bnb_{l}", tag="bnb")
                nc.vector.tensor_scalar(bnb[:], s_[:, 4:5], s_[:, 0:1],
                                        bcol(17 + l), ALU.mult, ALU.add)     # beta' = alpha*(-mu)+beta

                hT[l + 1] = wp.tile([H, N_LOC], bf16, name=f"hT{l + 1}", tag="hT", bufs=3)
                nc.scalar.activation(hT[l + 1][:], z2T[:], ACTF.Relu,
                                     bias=bnb[:], scale=s_[:, 4:5])
                if l + 1 < L:
                    update_h_all(hT[l + 1], l + 1)

            # ---- final stage ----
            h4 = hT[L]
            # B node-major
            Bn = bigp.tile([128, 512], bf16, name="Bn", tag="Bn")
            for ch in range(4):
                psb = psS.tile([128, H], f32, name="ps_bn", tag="psS")
                nc.tensor.matmul(psb[:], h4[:, ch * H:(ch + 1) * H], wmlp[:, H:2 * H])
                nc.scalar.activation(Bn[:, ch * H:(ch + 1) * H], psb[:], ACTF.Copy)

            for b8 in range(NB // 8):
                g = b8
                for pp in range(4):
                    pair = b8 * 4 + 4 + pp
                    if pair < NB // 2:
                        prefetch_eib(pair)
                # phase A: A' rows into EC_all rows BS..BS+8 (PE placement)
                for i in range(8):
                    b = b8 * 8 + i
                    psa = psS.tile([128, H], f32, name="ps_a8", tag="psS")
                    nc.tensor.matmul(psa[BS:BS + 8, :],
                                     h4[:, g * NP + i * 8:g * NP + i * 8 + 8],
                                     wmlp[:, 0:H], tile_position=(0, 96))
                    nc.vector.tensor_copy(EC_all[BS:BS + 8, b * H:(b + 1) * H],
                                          psa[BS:BS + 8, :])
                # phase B: dense matmul stream + relus
                rts = []
                for i in range(8):
                    b = b8 * 8 + i
                    pt = psA.tile([128, 512], f32, name="ps_T", tag="psA")
                    rb = (g % 2) * NP
                    nc.tensor.matmul(pt[:], Bn[rb:rb + NP, (g // 2) * H:(g // 2 + 1) * H],
                                     sb["rjx"][rb:rb + NP, :], start=True, stop=False)
                    nc.tensor.matmul(pt[:], EC_all[0:BS + 9, b * H:(b + 1) * H],
                                     eg[b // 2][:, b % 2, :], start=False, stop=True)
                    relu_t = fp.tile([128, 512], bf16, name="relu_t", tag="relu_t",
                                     bufs=9)
                    if b % 2 == 0:
                        nc.scalar.activation(relu_t[:], pt[:], ACTF.Relu)
                    else:
                        nc.vector.tensor_scalar_max(relu_t[:], pt[:], 0.0)
                    rts.append(relu_t)
                # phase C: w2 reductions (col-tiled groups of 4) + output
                for half in range(2):
                    po = psO.tile([128, 512], f32, name="ps_o", tag="psO")
                    for i in range(4):
                        nc.tensor.matmul(po[32 * i:32 * i + 1, :], sb["w2c"][:],
                                         rts[half * 4 + i][:],
                                         tile_position=(0, 32 * i))
                    stage = fp.tile([128, 512], f32, name="ostage", tag="ostage")
                    nc.scalar.activation(stage[:], po[:], ACTF.Copy,
                                         bias=mlp_b2_val)
                    nc.sync.dma_start(y[b8 * 8 + half * 4:b8 * 8 + half * 4 + 4, :],
                                      stage[0:97:32, :])

    _split_multi_waits(nc)
    return nc


def _split_multi_waits(nc, cap=1):
    """This walrus build accepts at most one sync wait per instruction; move
    extra waits onto same-engine NoOps inserted immediately before."""
    for fn in nc.m.functions:
        for bb in fn.blocks:
            out = []
            for inst in bb.instructions:
                si = inst.sync_info
                waits = list(si.on_wait) if si and si.on_wait else []
                if len(waits) > cap:
                    for w in waits[:-cap]:
                        nop = mybir.InstNoOp(
                            name=nc.get_next_instruction_name(),
                            sync_info=mybir.SyncInfo(on_wait=[w], on_update=[]),
                            bass_nofuse=True,
                            engine=inst.engine,
                        )
                        out.append(nop)
                    si.on_wait = waits[-cap:]
                out.append(inst)
            bb.instructions = out


# ---------------------------------------------------------------------------
# entry point
# ---------------------------------------------------------------------------

def kernel(**inputs):
    x = np.asarray(inputs["x"])
    edge_attr = np.asarray(inputs["edge_attr"])
    ei = np.asarray(inputs["edge_index"])
    src, dst = ei[0], ei[1]

    shared, b2val = _prep_shared(
        inputs["atom_W"], inputs["atom_b"], inputs["bond_W"], inputs["bond_b"],
        inputs["gbm_W1"], inputs["gbm_b1"], inputs["gbm_W2"], inputs["gbm_b2"],
        inputs["gnn_W1"], inputs["gnn_b1"], inputs["gnn_W2"], inputs["gnn_b2"],
        inputs["bn_gamma"], inputs["bn_beta"], inputs["mlp_W1"], inputs["mlp_b1"],
        inputs["mlp_W2"], inputs["mlp_b2"])

    in_maps = []
    for c in range(N_CORES):
        m = _prep_core(c, x, edge_attr, src, dst, shared)
        in_maps.append({k: np.ascontiguousarray(v) for k, v in m.items()})

    nc = build_program(b2val)
    trace = bool(int(os.environ.get("KERNEL_TRACE", "0")))
    res = run_bass_kernel_spmd(nc, in_maps, list(range(N_CORES)), trace=trace)
    kernel.last_exec_time_ns = res.exec_time_ns
    kernel.last_trace = res.instructions_and_trace

    out = np.concatenate([res.results[c]["y"].reshape(-1) for c in range(N_CORES)])
    return out.reshape(G * NP * NP, 1).astype(np.float32)


kernel.last_exec_time_ns = None
kernel.last_trace = None


# revision 17
# speedup vs baseline: 1.1243x; 1.0263x over previous
"""Trainium2 Bass kernel for nn_LinearEmbed (GINE message passing + all-pairs edge embed).

Sharding: data-parallel over graphs. 64 graphs -> 8 cores x 8 graphs.
Cross-core coupling: batchnorm statistics (AllReduce of [128,2] per layer).

Layout conventions (per core, G_loc=8 graphs, 512 nodes, 4096 edges):
  feature-major: [H=128 partitions, rows free]  (hT, u1T, ...)
  edge-major:    [128 edge partitions, H free]  (messages m, negu2, ec)
All matmuls in bf16 (f32 PSUM accumulate).

v2 notes:
  - bond encoder folded into u1 / ec weights on host (wu1 = bond_W @ gbm_W1,
    w3e = bond_W @ mlp_W1[2H:3H] + bias row): eT / eTb never materialized.
  - all u1 layers + negated-u2 (edge MLP part of messages) + ec emitted
    inside layer-0's collective window; layers 1-3 then only run the
    h-dependent gather matmuls and a 2-op relu  m = max(g, -u2) - (-u2).
  - const DMAs split across the SP / Activation / Pool DGE queues.
  - eib selector DMAs batched per 8-band group on the Pool queue.
  - per-band A' transposes replaced by one node-major A' (4 matmuls) +
    cross-partition vector copies.
"""

import os
import numpy as np
import ml_dtypes

import concourse.bass as bass
import concourse.mybir as mybir
import concourse.tile as tile
from concourse.vector_clock import ScopedClock
from concourse.bass_utils import run_bass_kernel_spmd

# problem constants
G, NP, EP, H = 64, 64, 512, 128
IN_F, EDGE_F, L = 32, 16, 4
BN_EPS = 1e-5
N_CORES = 8
G_LOC = G // N_CORES          # 8 graphs per core
N_LOC = G_LOC * NP            # 512 nodes
E_LOC = G_LOC * EP            # 4096 edges
NB = G_LOC * 8                # 64 bands per core (8 i-bands per graph)
BS = 96                       # band slot budget (max edges per band)
NTOT = float(G * NP)          # batchnorm population

f32 = mybir.dt.float32
bf16 = mybir.dt.bfloat16
AX = mybir.AxisListType
ALU = mybir.AluOpType
ACTF = mybir.ActivationFunctionType

bf = ml_dtypes.bfloat16


def _to_bf16(a):
    return np.asarray(a, dtype=np.float32).astype(bf)


class _SplitDrainTC(tile.TileContext):
    """Tail drain in this walrus build accepts only one sync wait; split the
    global-clock waits across multiple drain instructions."""

    def _drain_and_barrier(self, tick_clock, wait_clock):
        drain_inst = self.nc.sync.drain()
        wait_clock.add_sem_waits(
            drain_inst.ins, ScopedClock({None: tick_clock.global_clock})
        )
        si = drain_inst.ins.sync_info
        waits = list(si.on_wait or [])
        if len(waits) > 1:
            si.on_wait = [waits[0]]
            for w in waits[1:]:
                extra = self.nc.sync.drain()
                extra.ins.sync_info = mybir.SyncInfo(on_wait=[w], on_update=[])
        self.nc.all_engine_barrier()
        assert self.sems is not None
        popped = self.nc._tile_sem_poison_stack.pop()
        assert popped is self._sem_poison
        self.nc.clear_and_free_semaphores(list(self.sems.allocated().values()))


# ---------------------------------------------------------------------------
# host-side preprocessing: shard + sort + one ndarray per SBUF constant
# ---------------------------------------------------------------------------

def _prep_core(c, x, edge_attr, src, dst, weights):
    g0 = c * G_LOC
    lo, hi = g0 * NP, (g0 + G_LOC) * NP
    mask = (src >= lo) & (src < hi)
    esel = np.nonzero(mask)[0]
    assert ((dst[esel] >= lo) & (dst[esel] < hi)).all(), "cross-shard edge"

    # stable sort local edges by (graph, band)
    s_loc = src[esel] - lo
    band_key = (s_loc // NP) * 8 + (s_loc % NP) // 8
    order = np.argsort(band_key, kind="stable")
    esel = esel[order]
    s_loc = src[esel] - lo
    d_loc = dst[esel] - lo
    gl = s_loc // NP
    si = s_loc % NP
    di = d_loc % NP
    assert len(esel) == E_LOC, f"core {c}: {len(esel)} edges"
    assert (np.bincount(gl, minlength=G_LOC) == EP).all()

    ea = np.asarray(edge_attr)[esel]          # [E_LOC, 16] sorted order

    # gather matrix (+ ones row for the gbm_b2 bias trick): [65, 8*512]
    gmat = np.zeros((NP + 1, E_LOC), np.float32)
    gmat[si, np.arange(E_LOC)] = 1.0
    gmat[NP, :] = 1.0
    # scatter matrix chunks: [128, 32*64]; chunk (g,ch) -> cols (g*4+ch)*64
    smat = np.zeros((128, E_LOC // 128 * NP), np.float32)
    for g in range(G_LOC):
        for ch in range(EP // 128):
            sel = slice(g * EP + ch * 128, g * EP + (ch + 1) * 128)
            blk = np.zeros((128, NP), np.float32)
            blk[np.arange(128), di[sel]] = 1.0
            smat[:, (g * 4 + ch) * NP:(g * 4 + ch + 1) * NP] = blk

    # banded (padded) edge layout for the final stage; host-built Eind
    eab = np.zeros((NB * BS, EDGE_F), np.float32)
    eib = np.zeros((NB, BS + 9, 512), np.float32)
    eib[:, BS:BS + 8, :] = (np.arange(512) // NP == np.arange(8)[:, None])
    eib[:, BS + 8, :] = 1.0
    bands = gl * 8 + si // 8
    for b in range(NB):
        sel = np.nonzero(bands == b)[0]
        nb = len(sel)
        assert nb <= BS, f"band {b} has {nb} edges > {BS}"
        eab[b * BS:b * BS + nb] = ea[sel]
        eib[b, np.arange(nb), (si[sel] % 8) * NP + di[sel]] = 1.0

    # banded ea^T + ones row (bias trick for the folded bond bias)
    eaTbe = np.concatenate(
        [eab.T, np.ones((1, NB * BS), np.float32)], 0)   # [17, 6144]

    xc = np.asarray(x)[lo:hi]                  # [512, 32]

    out = {
        "xT": _to_bf16(xc.T),                                    # [32, 512]
        "eaT": _to_bf16(ea.T),                                   # [16, 4096]
        "eaTbe": _to_bf16(eaTbe),                                # [17, 6144]
        "gmat": _to_bf16(gmat),                                  # [65, 4096]
        "smat": _to_bf16(smat),                                  # [128, 2048]
        "eib": _to_bf16(eib),                                    # [64, 105, 512]
    }
    out.update(weights)
    return out


def _prep_shared(atom_W, atom_b, bond_W, bond_b, gbm_W1, gbm_b1, gbm_W2,
                 gbm_b2, gnn_W1, gnn_b1, gnn_W2, gnn_b2, bn_gamma, bn_beta,
                 mlp_W1, mlp_b1, mlp_W2, mlp_b2):
    bW = np.asarray(bond_W, np.float32)
    bb = np.asarray(bond_b, np.float32)
    gbW1 = np.asarray(gbm_W1, np.float32)
    gbb1 = np.asarray(gbm_b1, np.float32)
    gbW2 = np.asarray(gbm_W2, np.float32)
    W1 = np.asarray(mlp_W1, np.float32)
    W3 = W1[2 * H:3 * H]

    # bond encoder folded into the per-layer edge-MLP first layers
    wu1 = np.concatenate([bW @ gbW1[l] for l in range(L)], 1)    # [16, 512]
    bt1 = np.stack([bb @ gbW1[l] + gbb1[l] for l in range(L)])   # [4, 128]
    # ... and into the final-stage dense-C projection (+ bias row)
    w3e = np.concatenate([bW @ W3, (bb @ W3)[None, :]], 0)       # [17, 128]

    # block 0: +gbm_W2[0] (layer-0 inline u2); blocks 1-3: -gbm_W2[l]
    blocks = [gbW2[0], -gbW2[1], -gbW2[2], -gbW2[3]]
    blocks += [np.asarray(gnn_W1)[l] for l in range(L)]
    blocks += [np.asarray(gnn_W2)[l] for l in range(L)]
    wsb = np.concatenate(blocks, 1)                              # [128, 1536]

    wmlp_sb = np.concatenate([W1[0:H], W1[H:2 * H]], 1)          # [128, 256]

    bcols = np.zeros((H, 21), np.float32)
    bcols[:, 0] = np.asarray(atom_b)
    bcols[:, 1:5] = bt1.T
    bcols[:, 5:9] = np.asarray(gnn_b1).T
    bcols[:, 9:13] = np.asarray(gnn_b2).T
    bcols[:, 13:17] = np.asarray(bn_gamma).T
    bcols[:, 17:21] = np.asarray(bn_beta).T

    b2rep = np.tile(np.asarray(gbm_b2), (1, G_LOC))              # [4, 1024]

    rjx = np.zeros((128, 512), np.float32)
    j_of_p = np.arange(512) % NP
    for n in range(NP):
        rjx[n, j_of_p == n] = 1.0
        rjx[NP + n, j_of_p == n] = 1.0

    b1row = np.tile(np.asarray(mlp_b1)[None, :], (1, G_LOC * 8))  # [1, 8192]

    return {
        "b1row": _to_bf16(b1row),               # [1, 8192]
        "wA": _to_bf16(atom_W),                 # [32, 128]
        "wu1": _to_bf16(wu1),                   # [16, 512]
        "w3e": _to_bf16(w3e),                   # [17, 128]
        "wsb": _to_bf16(wsb),                   # [128, 1536]
        "wmlp": _to_bf16(wmlp_sb),              # [128, 256]
        "w2c": _to_bf16(np.asarray(mlp_W2)),    # [128, 1]
        "bcols": bcols,                         # [128, 21] f32
        "b2rep": _to_bf16(b2rep),               # [4, 1024]
        "rjx": _to_bf16(rjx),                   # [128, 512]
        "ident": _to_bf16(np.eye(128)),         # [128, 128]
    }, float(np.asarray(mlp_b2)[0])


# ---------------------------------------------------------------------------
# device program
# ---------------------------------------------------------------------------

INPUT_SPECS = {
    "xT": ([IN_F, N_LOC], bf16), "eaT": ([EDGE_F, E_LOC], bf16),
    "eaTbe": ([EDGE_F + 1, NB * BS], bf16), "gmat": ([NP + 1, E_LOC], bf16),
    "smat": ([128, 32 * NP], bf16), "eib": ([NB, BS + 9, 512], bf16),
    "wA": ([IN_F, H], bf16), "wu1": ([EDGE_F, L * H], bf16),
    "w3e": ([EDGE_F + 1, H], bf16),
    "wsb": ([H, 12 * H], bf16), "wmlp": ([H, 2 * H], bf16),
    "w2c": ([H, 1], bf16), "bcols": ([H, 21], f32),
    "b2rep": ([L, G_LOC * H], bf16), "rjx": ([128, 512], bf16),
    "ident": ([128, 128], bf16), "b1row": ([1, NB * H], bf16),
}

# DMA issue queues for the constants, in issue order
_SYNC_CONSTS = ["eaT", "gmat", "xT", "b2rep", "rjx"]
_SCAL_CONSTS = ["wu1", "bcols", "wA", "wsb", "smat", "wmlp", "w2c", "ident"]
_POOL_CONSTS = ["eaTbe", "w3e"]


def build_program(mlp_b2_val):
    nc = bass.Bass(trn_type="TRN2", num_devices=N_CORES)
    dins = {n: nc.dram_tensor(n, shp, dt, kind="ExternalInput")
            for n, (shp, dt) in INPUT_SPECS.items()}
    y = nc.dram_tensor("y", [NB, 512], f32, kind="ExternalOutput")
    b2dram = dins["b2rep"]

    with _SplitDrainTC(nc) as tc:
        with tc.tile_pool(name="const", bufs=1) as cpool, \
             tc.tile_pool(name="big", bufs=1) as bigp, \
             tc.tile_pool(name="work", bufs=2) as wp, \
             tc.tile_pool(name="fin", bufs=3) as fp, \
             tc.tile_pool(name="egp", bufs=3) as egp, \
             tc.tile_pool(name="dram", bufs=1, space="DRAM") as dram, \
             tc.tile_pool(name="psA", bufs=4, space="PSUM") as psA, \
             tc.tile_pool(name="psS", bufs=2, space="PSUM") as psS, \
             tc.tile_pool(name="psO", bufs=2, space="PSUM") as psO:

            # absorb the ncfw startup barrier as early as possible: emit the
            # dummy AllReduce before anything else so its trigger leads the
            # gpsimd queue
            dmy_in = dram.tile([16, 2], f32, name="dmy_in")
            dmy_out = dram.tile([16, 2], f32, name="dmy_out", addr_space="Shared")
            nc.gpsimd.collective_compute(
                "AllReduce", ALU.add, replica_groups=[list(range(N_CORES))],
                ins=[dmy_in.opt()], outs=[dmy_out.opt()])

            # ---- load constants (three parallel DGE queues) ----
            sb = {}
            for n in _SYNC_CONSTS + _SCAL_CONSTS + _POOL_CONSTS:
                shp, dt = INPUT_SPECS[n]
                sb[n] = cpool.tile(shp, dt, name=f"c_{n}", tag=f"c_{n}")
            for n in _SYNC_CONSTS:
                if n == "eaT":
                    # split so the first u1 chunks start before the full
                    # 131KB transfer lands
                    half = E_LOC // 2
                    nc.sync.dma_start(sb[n][:, 0:half], dins[n][:, 0:half])
                    nc.sync.dma_start(sb[n][:, half:], dins[n][:, half:])
                else:
                    nc.sync.dma_start(sb[n][:], dins[n][:])
            for n in _SCAL_CONSTS:
                nc.scalar.dma_start(sb[n][:], dins[n][:])
            for n in _POOL_CONSTS:
                nc.gpsimd.dma_start(sb[n][:], dins[n][:])
            bc = sb["bcols"]

            def bcol(i):
                return bc[:, i:i + 1]

            wsb, wmlp = sb["wsb"], sb["wmlp"]

            def wsq(i):
                return wsb[:, i * H:(i + 1) * H]

            # ---- ec rows container for all bands (final-stage C path) ----
            EC_all = bigp.tile([128, NB * H], bf16, name="EC_all", tag="EC_all")
            nc.gpsimd.dma_start(EC_all[BS + 8:BS + 9, :], dins["b1row"][:])

            # ---- u1[l] = relu(ea @ wu1[l] + bt1[l])  (bond encoder folded) ----
            u1 = [None] * L

            def emit_u1(l):
                u1[l] = wp.tile([H, E_LOC], bf16, name=f"u1_{l}", tag="u1", bufs=4)
                for j in range(E_LOC // 512):
                    pool, tg = (psA, "psA") if j % 2 == 0 else (psO, "psO")
                    ps = pool.tile([128, 512], f32, name="ps_u1", tag=tg)
                    nc.tensor.matmul(ps[:], sb["wu1"][:, l * H:(l + 1) * H],
                                     sb["eaT"][:, j * 512:(j + 1) * 512])
                    nc.scalar.activation(u1[l][:, j * 512:(j + 1) * 512], ps[:],
                                         ACTF.Relu, bias=bcol(1 + l))

            # ---- negu2[l] = -(u1[l] @ gbm_W2[l]) edge-major, for layers 1-3 ----
            negu2 = [None] * L

            def emit_negu2(l):
                negu2[l] = wp.tile([128, E_LOC], bf16, name=f"nu2_{l}",
                                   tag="nu2", bufs=3)
                for j in range(8):
                    pool, tg = (psA, "psA") if j % 2 == 0 else (psO, "psO")
                    ps = pool.tile([128, 512], f32, name="ps_nu2", tag=tg)
                    for ch in range(4):
                        e0 = j * 512 + ch * 128
                        nc.tensor.matmul(ps[:, ch * H:(ch + 1) * H],
                                         u1[l][:, e0:e0 + 128], wsq(l))
                    dst = negu2[l][:, j * 512:(j + 1) * 512]
                    if j % 2 == 0:
                        nc.scalar.activation(dst, ps[:], ACTF.Copy)
                    else:
                        nc.vector.tensor_copy(dst, ps[:])

            # ---- ec rows for all bands (h-independent; fills window 0) ----
            def emit_ec():
                for b in range(NB):
                    pse = psS.tile([BS, H], f32, name="ps_ec", tag="psS")
                    nc.tensor.matmul(pse[:], sb["eaTbe"][:, b * BS:(b + 1) * BS],
                                     sb["w3e"][:])
                    dst = EC_all[0:BS, b * H:(b + 1) * H]
                    if b % 2 == 0:
                        nc.vector.tensor_copy(dst, pse[:])
                    else:
                        nc.scalar.activation(dst, pse[:], ACTF.Copy)

            # ---- node-major h (+bias row) via one XBAR transpose DMA ----
            h_all = bigp.tile([NP + 1, G_LOC * H], bf16, name="h_all", tag="h_all")

            def update_h_all(hts, l):
                nc.sync.dma_start(h_all[NP:NP + 1, :], b2dram[l:l + 1, :])
                for g in range(G_LOC):
                    pst = psS.tile([NP, H], f32, name="ps_ht", tag="psS")
                    nc.tensor.matmul(pst[:], hts[:, g * NP:(g + 1) * NP], sb["ident"][:])
                    dst = h_all[0:NP, g * H:(g + 1) * H]
                    if g % 2 == 0:
                        nc.scalar.activation(dst, pst[:], ACTF.Copy)
                    else:
                        nc.vector.tensor_copy(dst, pst[:])

            # ---- eib prefetch: pairs of bands, alternating DGE queues ----
            eg = {}

            def prefetch_eib(pair):
                t = egp.tile([BS + 9, 2, 512], bf16, name="egrp", tag="egrp",
                             bufs=10)
                src = dins["eib"][pair * 2:(pair + 1) * 2, 0:BS + 9, :]
                eng = nc.sync if pair % 2 == 0 else nc.scalar
                eng.dma_start(t[:, :, :], src.rearrange("i p c -> p i c"))
                eg[pair] = t

            # ---- atom encoder ----
            hT = [None] * (L + 1)
            emit_u1(0)
            hT[0] = wp.tile([H, N_LOC], bf16, name="hT0", tag="hT", bufs=3)
            ps = psA.tile([128, 512], f32, name="ps_h0", tag="psA")
            nc.tensor.matmul(ps[:], sb["wA"][:], sb["xT"][:])
            nc.vector.tensor_scalar_add(hT[0][:], ps[:], bcol(0))
            update_h_all(hT[0], 0)

            # ---- GINE layers ----
            for l in range(L):
                # messages, edge-major
                m_sb = wp.tile([128, E_LOC], bf16, name=f"m_{l}", tag="m")
                for g in range(G_LOC):
                    pool, tg = (psA, "psA") if g % 2 == 0 else (psO, "psO")
                    psm = pool.tile([128, 512], f32, name="ps_m", tag=tg)
                    if l == 0:
                        for ch in range(4):
                            e0 = g * EP + ch * 128
                            nc.tensor.matmul(psm[:, ch * H:(ch + 1) * H],
                                             u1[0][:, e0:e0 + 128],
                                             wsq(0), start=True, stop=False)
                            nc.tensor.matmul(psm[:, ch * H:(ch + 1) * H],
                                             sb["gmat"][0:NP + 1, e0:e0 + 128],
                                             h_all[0:NP + 1, g * H:(g + 1) * H],
                                             start=False, stop=True)
                        for ch in range(4):
                            dst = m_sb[:, (g * 4 + ch) * H:(g * 4 + ch + 1) * H]
                            src = psm[:, ch * H:(ch + 1) * H]
                            if ch % 2 == 0:
                                nc.scalar.activation(dst, src, ACTF.Relu)
                            else:
                                nc.vector.tensor_scalar_max(dst, src, 0.0)
                    else:
                        for ch in range(4):
                            e0 = g * EP + ch * 128
                            nc.tensor.matmul(psm[:, ch * H:(ch + 1) * H],
                                             sb["gmat"][0:NP + 1, e0:e0 + 128],
                                             h_all[0:NP + 1, g * H:(g + 1) * H])
                        # m = relu(gather + u2) = max(gather, -u2) - (-u2)
                        nsl = negu2[l][:, g * 512:(g + 1) * 512]
                        tmp = wp.tile([128, 512], bf16, name="mtmp", tag="mtmp",
                                      bufs=4)
                        nc.vector.tensor_tensor(tmp[:], psm[:], nsl, ALU.max)
                        nc.gpsimd.tensor_tensor(m_sb[:, g * 512:(g + 1) * 512],
                                                tmp[:], nsl, ALU.subtract)

                # scatter-aggregate (feature-major out)
                psagg = psA.tile([128, 512], f32, name="ps_agg", tag="psA")
                for g in range(G_LOC):
                    for ch in range(4):
                        nc.tensor.matmul(psagg[:, g * NP:(g + 1) * NP],
                                         m_sb[:, (g * 4 + ch) * H:(g * 4 + ch + 1) * H],
                                         sb["smat"][:, (g * 4 + ch) * NP:(g * 4 + ch + 1) * NP],
                                         start=(ch == 0), stop=(ch == 3))
                zT = wp.tile([H, N_LOC], bf16, name=f"zT_{l}", tag="zT")
                nc.vector.tensor_tensor(zT[:], hT[l][:], psagg[:], ALU.add)

                # node MLP
                ps1 = psA.tile([128, 512], f32, name="ps_z1", tag="psA")
                nc.tensor.matmul(ps1[:], wsq(4 + l), zT[:])
                y1T = wp.tile([H, N_LOC], bf16, name=f"y1_{l}", tag="y1")
                nc.scalar.activation(y1T[:], ps1[:], ACTF.Relu, bias=bcol(5 + l))
                ps2 = psA.tile([128, 512], f32, name="ps_z2", tag="psA")
                nc.tensor.matmul(ps2[:], wsq(8 + l), y1T[:])
                z2T = wp.tile([H, N_LOC], f32, name=f"z2_{l}", tag="z2")
                nc.scalar.activation(z2T[:], ps2[:], ACTF.Identity, bias=bcol(9 + l))

                # batchnorm stats: S on vector, Q on scalar (Square + accum)
                st = wp.tile([H, 2], f32, name=f"st_{l}", tag="st")
                nc.vector.reduce_sum(st[:, 0:1], z2T[:], axis=AX.X)
                sq = wp.tile([H, N_LOC], f32, name=f"sq_{l}", tag="sq")
                nc.scalar.activation(sq[:], z2T[:], ACTF.Square,
                                     accum_out=st[:, 1:2])
                cc_in = dram.tile([H, 2], f32, name=f"ccin_{l}")
                cc_out = dram.tile([H, 2], f32, name=f"ccout_{l}", addr_space="Shared")
                nc.sync.dma_start(cc_in[:], st[:])
                nc.gpsimd.collective_compute(
                    "AllReduce", ALU.add,
                    replica_groups=[list(range(N_CORES))],
                    ins=[cc_in.opt()], outs=[cc_out.opt()])

                # fill the collective window with h-independent work
                if l == 0:
                    emit_u1(1)
                    emit_negu2(1)
                    emit_ec()
                elif l == 1:
                    emit_u1(2)
                    emit_negu2(2)
                elif l == 2:
                    emit_u1(3)
                    emit_negu2(3)
                    prefetch_eib(0)
                    prefetch_eib(1)
                elif l == 3:
                    prefetch_eib(2)
                    prefetch_eib(3)

                st2 = wp.tile([H, 2], f32, name=f"st2_{l}", tag="st2")
                nc.sync.dma_start(st2[:], cc_out[:])

                # alpha = gamma*rsqrt(var+eps); beta' = beta - mu*alpha
                s_ = wp.tile([H, 6], f32, name=f"bn_{l}", tag="bn")
                nc.vector.tensor_scalar_mul(s_[:, 0:1], st2[:, 0:1], -1.0 / NTOT)   # -mu
                nc.vector.tensor_scalar(s_[:, 1:2], s_[:, 0:1], st2[:, 0:1],
                                        BN_EPS * NTOT, ALU.mult, ALU.add)  # -S^2/N + N*eps
                nc.vector.tensor_tensor(s_[:, 2:3], s_[:, 1:2], st2[:, 1:2], ALU.add)  # +Q
                nc.scalar.activation(s_[:, 3:4], s_[:, 2:3], ACTF.Sqrt,
                                     scale=1.0 / NTOT)                 # sqrt(var+eps)
                nc.vector.reciprocal(s_[:, 5:6], s_[:, 3:4])
                nc.vector.tensor_tensor(s_[:, 4:5], s_[:, 5:6], bcol(13 + l), ALU.mult)  # alpha
                bnb = wp.tile([H, 1], f32, name=f"bnb_{l}", tag="bnb")
                nc.vector.tensor_scalar(bnb[:], s_[:, 4:5], s_[:, 0:1],
                                        bcol(17 + l), ALU.mult, ALU.add)     # beta' = alpha*(-mu)+beta

                hT[l + 1] = wp.tile([H, N_LOC], bf16, name=f"hT{l + 1}", tag="hT", bufs=3)
                nc.scalar.activation(hT[l + 1][:], z2T[:], ACTF.Relu,
                                     bias=bnb[:], scale=s_[:, 4:5])
                if l + 1 < L:
                    update_h_all(hT[l + 1], l + 1)

            # ---- final stage ----
            h4 = hT[L]
            # B node-major
            Bn = bigp.tile([128, 512], bf16, name="Bn", tag="Bn")
            for ch in range(4):
                psb = psS.tile([128, H], f32, name="ps_bn", tag="psS")
                nc.tensor.matmul(psb[:], h4[:, ch * H:(ch + 1) * H], wmlp[:, H:2 * H])
                nc.scalar.activation(Bn[:, ch * H:(ch + 1) * H], psb[:], ACTF.Copy)

            for b8 in range(NB // 8):
                g = b8
                for pp in range(4):
                    pair = b8 * 4 + 4 + pp
                    if pair < NB // 2:
                        prefetch_eib(pair)
                # phase A: A' rows into EC_all rows BS..BS+8 (PE placement)
                for i in range(8):
                    b = b8 * 8 + i
                    psa = psS.tile([128, H], f32, name="ps_a8", tag="psS")
                    nc.tensor.matmul(psa[BS:BS + 8, :],
                                     h4[:, g * NP + i * 8:g * NP + i * 8 + 8],
                                     wmlp[:, 0:H], tile_position=(0, 96))
                    nc.vector.tensor_copy(EC_all[BS:BS + 8, b * H:(b + 1) * H],
                                          psa[BS:BS + 8, :])
                # phase B: dense matmul stream + relus
                rts = []
                for i in range(8):
                    b = b8 * 8 + i
                    pt = psA.tile([128, 512], f32, name="ps_T", tag="psA")
                    rb = (g % 2) * NP
                    nc.tensor.matmul(pt[:], Bn[rb:rb + NP, (g // 2) * H:(g // 2 + 1) * H],
                                     sb["rjx"][rb:rb + NP, :], start=True, stop=False)
                    nc.tensor.matmul(pt[:], EC_all[0:BS + 9, b * H:(b + 1) * H],
                                     eg[b // 2][:, b % 2, :], start=False, stop=True)
                    relu_t = fp.tile([128, 512], bf16, name="relu_t", tag="relu_t",
                                     bufs=9)
                    if b % 2 == 0:
                        nc.scalar.activation(relu_t[:], pt[:], ACTF.Relu)
                    else:
                        nc.vector.tensor_scalar_max(relu_t[:], pt[:], 0.0)
                    rts.append(relu_t)
                # phase C: w2 reductions (col-tiled groups of 4) + output
                for half in range(2):
                    po = psO.tile([128, 512], f32, name="ps_o", tag="psO")
                    for i in range(4):
                        nc.tensor.matmul(po[32 * i:32 * i + 1, :], sb["w2c"][:],
                                         rts[half * 4 + i][:],
                                         tile_position=(0, 32 * i))
                    stage = fp.tile([128, 512], f32, name="ostage", tag="ostage")
                    nc.scalar.activation(stage[:], po[:], ACTF.Copy,
                                         bias=mlp_b2_val)
                    nc.sync.dma_start(y[b8 * 8 + half * 4:b8 * 8 + half * 4 + 4, :],
                                      stage[0:97:32, :])

    _split_multi_waits(nc)
    return nc


def _split_multi_waits(nc, cap=1):
    """This walrus build accepts at most one sync wait per instruction; move
    extra waits onto same-engine NoOps inserted immediately before."""
    for fn in nc.m.functions:
        for bb in fn.blocks:
            out = []
            for inst in bb.instructions:
                si = inst.sync_info
                waits = list(si.on_wait) if si and si.on_wait else []
                if len(waits) > cap:
                    for w in waits[:-cap]:
                        nop = mybir.InstNoOp(
                            name=nc.get_next_instruction_name(),
                            sync_info=mybir.SyncInfo(on_wait=[w], on_update=[]),
                            bass_nofuse=True,
                            engine=inst.engine,
                        )
                        out.append(nop)
                    si.on_wait = waits[-cap:]
                out.append(inst)
            bb.instructions = out


# ---------------------------------------------------------------------------
# entry point
# ---------------------------------------------------------------------------

def kernel(**inputs):
    x = np.asarray(inputs["x"])
    edge_attr = np.asarray(inputs["edge_attr"])
    ei = np.asarray(inputs["edge_index"])
    src, dst = ei[0], ei[1]

    shared, b2val = _prep_shared(
        inputs["atom_W"], inputs["atom_b"], inputs["bond_W"], inputs["bond_b"],
        inputs["gbm_W1"], inputs["gbm_b1"], inputs["gbm_W2"], inputs["gbm_b2"],
        inputs["gnn_W1"], inputs["gnn_b1"], inputs["gnn_W2"], inputs["gnn_b2"],
        inputs["bn_gamma"], inputs["bn_beta"], inputs["mlp_W1"], inputs["mlp_b1"],
        inputs["mlp_W2"], inputs["mlp_b2"])

    in_maps = []
    for c in range(N_CORES):
        m = _prep_core(c, x, edge_attr, src, dst, shared)
        in_maps.append({k: np.ascontiguousarray(v) for k, v in m.items()})

    nc = build_program(b2val)
    trace = bool(int(os.environ.get("KERNEL_TRACE", "0")))
    res = run_bass_kernel_spmd(nc, in_maps, list(range(N_CORES)), trace=trace)
    kernel.last_exec_time_ns = res.exec_time_ns
    kernel.last_trace = res.instructions_and_trace

    out = np.concatenate([res.results[c]["y"].reshape(-1) for c in range(N_CORES)])
    return out.reshape(G * NP * NP, 1).astype(np.float32)


kernel.last_exec_time_ns = None
kernel.last_trace = None
